# revision 13
# baseline (speedup 1.0000x reference)
"""CPC loss kernel for Trainium2, 8 NeuronCores, batch-sharded SPMD.

Pipeline per core (batch shard of 32):
  conv1d (as matmul over im2col'd input) -> enc_T [256, 140, 32] in SBUF
  GRU 128 steps (gate-on-partition layout; input projection gi precomputed
  in 32-step chunks that fill PE gaps during the recurrence)
  AllGather h_last across the 8 cores
  per prediction head k: pred matmul, then sim matmuls (rows = local
  (s, b) pairs, all 256 contrastive columns) fused with softmax stats:
  DVE max-reduce + ACT exp(accum_out) straight from PSUM.
Outputs per core: max/sumexp partials over local rows, diagonal logits,
hidden-state shard. Final (accuracy, loss, hidden) assembled on host.
"""

import sys

sys.path.insert(0, "/opt/trn_rl_repo")

import numpy as np

import concourse.bass as bass
import concourse.tile as tile
from concourse import bacc, mybir
from concourse.bass_utils import run_bass_kernel_spmd

F32 = mybir.dt.float32
BF16 = mybir.dt.bfloat16
AF = mybir.ActivationFunctionType
ALU = mybir.AluOpType

NCORES = 8
B, L, C = 256, 2048, 12
T_IN, T_OUT, H, STRIDE = 128, 12, 256, 4
S = T_IN + T_OUT              # 140 steps used
BL = B // NCORES              # 32 batch per core
ROWS = S * BL                 # 4480 local sim rows
KW = C * STRIDE               # 48 im2col contraction
G = 3 * H                     # 768 gate width
GCH = G // 128                # 6 gate chunks
LCH = H // 128                # 2 latent chunks
EXP_SHIFT = -20.0             # exp(x + EXP_SHIFT); added back on host

import os

PHASES = int(os.environ.get("KERNEL_PHASES", "4"))  # debug bisect knob
SIMSUB = int(os.environ.get("KERNEL_SIMSUB", "15"))  # bit0 pk, bit1 dk, bit2 mm+max, bit3 exp

_CACHE = {}


def _flat(ap):
    # [128, d0, d1] -> [128, d0*d1]
    return ap.rearrange("p a b -> p (a b)")


def _build():
    nc = bacc.Bacc(
        "TRN2",
        target_bir_lowering=False,
        debug=False,
        enable_asserts=False,
        num_devices=NCORES,
    )

    # ---- per-core inputs (host pre-laid-out, see kernel()) ----
    xT = nc.dram_tensor("xT", [KW, S, BL], F32, kind="ExternalInput")
    wr = nc.dram_tensor("wr", [KW, H], F32, kind="ExternalInput")
    cb = nc.dram_tensor("cb", [128, LCH], F32, kind="ExternalInput")
    wih = nc.dram_tensor("wih", [128, 2, GCH, 128], F32, kind="ExternalInput")
    whh = nc.dram_tensor("whh", [128, 2, GCH, 128], F32, kind="ExternalInput")
    gib = nc.dram_tensor("gib", [128, GCH], F32, kind="ExternalInput")
    bhn = nc.dram_tensor("bhn", [128, 2], F32, kind="ExternalInput")
    pw = nc.dram_tensor("pw", [128, T_OUT, 2, 2, 128], F32, kind="ExternalInput")
    pb = nc.dram_tensor("pb", [128, T_OUT, 2], F32, kind="ExternalInput")
    id32 = nc.dram_tensor("id32", [BL, BL], F32, kind="ExternalInput")

    # ---- per-core outputs ----
    out_m = nc.dram_tensor("out_m", [128, 2, T_OUT], F32, kind="ExternalOutput")
    out_s = nc.dram_tensor("out_s", [128, 2, T_OUT], F32, kind="ExternalOutput")
    out_diag = nc.dram_tensor("out_diag", [BL, T_OUT], F32, kind="ExternalOutput")
    out_h = nc.dram_tensor("out_h", [128, LCH, BL], F32, kind="ExternalOutput")

    # collective buffers
    h_gath = nc.dram_tensor(
        "h_gath", [NCORES, 128, LCH, BL], F32, addr_space="Shared"
    )

    from contextlib import ExitStack

    with tile.TileContext(nc) as tc, ExitStack() as ctx:
        consts = ctx.enter_context(tc.tile_pool(name="consts", bufs=1))
        bigs = ctx.enter_context(tc.tile_pool(name="bigs", bufs=1))
        gip = ctx.enter_context(tc.tile_pool(name="gip", bufs=3))
        hp = ctx.enter_context(tc.tile_pool(name="hp", bufs=3))
        gp = ctx.enter_context(tc.tile_pool(name="gates", bufs=3))
        stp = ctx.enter_context(tc.tile_pool(name="stats", bufs=1))
        pkp = ctx.enter_context(tc.tile_pool(name="pkp", bufs=2))
        dmp = ctx.enter_context(tc.tile_pool(name="dump", bufs=2))
        drp = ctx.enter_context(tc.tile_pool(name="dram", bufs=1, space="DRAM"))

        # ---- load constants ----
        x_sb = consts.tile([KW, S * BL], F32, tag="x")
        nc.sync.dma_start(out=x_sb, in_=xT.ap().rearrange("p s b -> p (s b)"))
        wr_sb = consts.tile([KW, H], F32, tag="wr")
        nc.sync.dma_start(out=wr_sb, in_=wr.ap())
        cb_sb = consts.tile([128, LCH], F32, tag="cb")
        nc.sync.dma_start(out=cb_sb, in_=cb.ap())
        wih_sb = consts.tile([128, 2, GCH, 128], F32, tag="wih")
        nc.sync.dma_start(out=wih_sb, in_=wih.ap())
        whh_sb = consts.tile([128, 2, GCH, 128], F32, tag="whh")
        nc.sync.dma_start(out=whh_sb, in_=whh.ap())
        gib_sb = consts.tile([128, GCH], F32, tag="gib")
        nc.sync.dma_start(out=gib_sb, in_=gib.ap())
        bhn_sb = consts.tile([128, 2], F32, tag="bhn")
        nc.sync.dma_start(out=bhn_sb, in_=bhn.ap())
        pw_sb = consts.tile([128, T_OUT, 2, 2, 128], F32, tag="pw")
        nc.sync.dma_start(out=pw_sb, in_=pw.ap())
        pb_sb = consts.tile([128, T_OUT, 2], F32, tag="pb")
        nc.sync.dma_start(out=pb_sb, in_=pb.ap())
        id_sb = consts.tile([BL, BL], F32, tag="id32")
        nc.sync.dma_start(out=id_sb, in_=id32.ap())
        shift_sb = consts.tile([128, 1], F32, tag="shift")
        nc.vector.memset(shift_sb, EXP_SHIFT)

        encT = bigs.tile([128, LCH, S, BL], F32, tag="encT")

        with tc.tile_pool(name="ps_a", bufs=2, space="PSUM") as ps_a, tc.tile_pool(
            name="ps_g", bufs=2, space="PSUM"
        ) as ps_g:
            # ---- conv: enc_T[lat, (s, b)] = wr.T @ x ----
            for mch in range(LCH):
                for j0 in range(0, S * BL, 512):
                    n = min(512, S * BL - j0)
                    ps = ps_a.tile([128, 512], F32, tag="ps_a")
                    nc.tensor.matmul(
                        out=ps[:, :n],
                        lhsT=wr_sb[:, mch * 128 : (mch + 1) * 128],
                        rhs=x_sb[:, j0 : j0 + n],
                        start=True,
                        stop=True,
                    )
                    nc.scalar.activation(
                        out=_flat(encT[:, mch])[:, j0 : j0 + n],
                        in_=ps[:, :n],
                        func=AF.Identity,
                        bias=cb_sb[:, mch : mch + 1],
                        scale=1.0,
                    )

            # ---- gi chunks: gi[g, ts, b] = W_ih.T-tiles @ enc chunk ----
            NGI = T_IN // 32  # 4 chunks of 32 steps
            gi_tiles = [None] * NGI

            def gi_chunk_gen(tch):
                t0 = tch * 32
                gt = gip.tile([128, GCH, 32, BL], F32, tag="gi")
                gi_tiles[tch] = gt
                for gch in range(GCH):
                    ps0 = ps_a.tile([128, 512], F32, tag="ps_a")
                    ps1 = ps_a.tile([128, 512], F32, tag="ps_a")
                    for kch in range(2):
                        nc.tensor.matmul(
                            out=ps0,
                            lhsT=wih_sb[:, kch, gch],
                            rhs=_flat(encT[:, kch, t0 : t0 + 16]),
                            start=(kch == 0),
                            stop=(kch == 1),
                        )
                        yield
                        nc.tensor.matmul(
                            out=ps1,
                            lhsT=wih_sb[:, kch, gch],
                            rhs=_flat(encT[:, kch, t0 + 16 : t0 + 32]),
                            start=(kch == 0),
                            stop=(kch == 1),
                        )
                        yield
                    nc.scalar.activation(
                        out=_flat(gt[:, gch, 0:16]),
                        in_=ps0,
                        func=AF.Identity,
                        bias=gib_sb[:, gch : gch + 1],
                        scale=1.0,
                    )
                    yield
                    nc.scalar.activation(
                        out=_flat(gt[:, gch, 16:32]),
                        in_=ps1,
                        func=AF.Identity,
                        bias=gib_sb[:, gch : gch + 1],
                        scale=1.0,
                    )
                    yield

            # chunks 0 and 1 up front
            for tch in range(2):
                for _ in gi_chunk_gen(tch):
                    pass

            # ---- GRU recurrence ----
            h = hp.tile([128, LCH, BL], F32, tag="h")
            nc.vector.memset(h, 0.0)
            pending = None
            for t in range(T_IN if PHASES >= 2 else 0):
                tch, ts = t // 32, t % 32
                if ts == 0 and tch + 2 < NGI:
                    pending = gi_chunk_gen(tch + 2)
                gt = gi_tiles[tch]

                psg = ps_g.tile([128, GCH, BL], F32, tag="ps_g")
                # order: r (0,1), n (4,5), z (2,3) for early r-chain start
                for gch in (0, 1, 4, 5, 2, 3):
                    for kch in range(2):
                        nc.tensor.matmul(
                            out=psg[:, gch],
                            lhsT=whh_sb[:, kch, gch],
                            rhs=h[:, kch],
                            start=(kch == 0),
                            stop=(kch == 1),
                        )
                # r = sigmoid(gi_r + gh_r)
                a_r = gp.tile([128, 2, BL], F32, tag="a_r")
                nc.vector.tensor_tensor(
                    out=a_r, in0=psg[:, 0:2], in1=gt[:, 0:2, ts], op=ALU.add
                )
                r = gp.tile([128, 2, BL], F32, tag="r")
                nc.scalar.activation(out=r, in_=a_r, func=AF.Sigmoid)
                # n = tanh(gi_n + r * (gh_n + b_hh_n))
                rhn = gp.tile([128, 2, BL], F32, tag="rhn")
                for ch in range(2):
                    nc.vector.scalar_tensor_tensor(
                        out=rhn[:, ch],
                        in0=psg[:, 4 + ch],
                        scalar=bhn_sb[:, ch : ch + 1],
                        in1=r[:, ch],
                        op0=ALU.add,
                        op1=ALU.mult,
                    )
                gn = gp.tile([128, 2, BL], F32, tag="gn")
                nc.vector.tensor_tensor(
                    out=gn, in0=rhn, in1=gt[:, 4:6, ts], op=ALU.add
                )
                n = gp.tile([128, 2, BL], F32, tag="n")
                nc.scalar.activation(out=n, in_=gn, func=AF.Tanh)
                # z = sigmoid(gi_z + gh_z)
                a_z = gp.tile([128, 2, BL], F32, tag="a_z")
                nc.vector.tensor_tensor(
                    out=a_z, in0=psg[:, 2:4], in1=gt[:, 2:4, ts], op=ALU.add
                )
                z = gp.tile([128, 2, BL], F32, tag="z")
                nc.scalar.activation(out=z, in_=a_z, func=AF.Sigmoid)
                # h' = n + z * (h - n)
                d = gp.tile([128, 2, BL], F32, tag="d")
                nc.vector.tensor_tensor(out=d, in0=h, in1=n, op=ALU.subtract)
                e = gp.tile([128, 2, BL], F32, tag="e")
                nc.vector.tensor_tensor(out=e, in0=z, in1=d, op=ALU.mult)
                h_new = hp.tile([128, LCH, BL], F32, tag="h")
                nc.vector.tensor_tensor(out=h_new, in0=e, in1=n, op=ALU.add)
                h = h_new

                # drip-feed next gi chunk instructions into the stream
                if pending is not None:
                    for _ in range(2):
                        if next(pending, "done") == "done":
                            pending = None
                            break

        # ---- gather h_last across cores ----
        nc.sync.dma_start(out=out_h.ap(), in_=h)
        h_full = bigs.tile([128, LCH, NCORES, BL], F32, tag="h_full")
        if PHASES >= 3:
            h_loc = drp.tile([128, LCH, BL], F32, tag="h_loc")
            nc.gpsimd.dma_start(out=h_loc, in_=h)
            nc.gpsimd.collective_compute(
                "AllGather",
                ALU.bypass,
                replica_groups=[list(range(NCORES))],
                ins=[h_loc.opt()],
                outs=[h_gath.ap()],
            )
            nc.sync.dma_start(
                out=h_full, in_=h_gath.ap().rearrange("c p l b -> p l c b")
            )
        else:
            nc.vector.memset(h_full, 0.0)

        # ---- per-head pred + sim + softmax stats ----
        mpart = stp.tile([128, 2, T_OUT, 9], F32, tag="mpart")
        spart = stp.tile([128, 2, T_OUT, 9], F32, tag="spart")
        m_sb = stp.tile([128, 2, T_OUT], F32, tag="m_sb")
        s_sb = stp.tile([128, 2, T_OUT], F32, tag="s_sb")
        diag_sb = stp.tile([BL, T_OUT], F32, tag="diag")
        dscr = stp.tile([BL, BL], F32, tag="dscr")

        nc.vector.memset(mpart, 0.0)
        nc.vector.memset(spart, 0.0)
        nc.vector.memset(m_sb, 0.0)
        nc.vector.memset(s_sb, 1.0)
        nc.vector.memset(diag_sb, 0.0)

        with tc.tile_pool(name="ps_sim", bufs=2, space="PSUM") as ps_sim, tc.tile_pool(
            name="ps_pk", bufs=1, space="PSUM"
        ) as ps_pk, tc.tile_pool(name="ps_dk", bufs=1, space="PSUM") as ps_dk:
            for k in range(T_OUT if PHASES >= 4 else 0):
                # P_k^T [lam, c] for all 256 c
                pkT = pkp.tile([128, 2, B], F32, tag="pkT")
                if not SIMSUB & 1:
                    nc.vector.memset(pkT, 0.001)
                for lch in range(2 if SIMSUB & 1 else 0):
                    psp = ps_pk.tile([128, B], F32, tag="ps_pk")
                    for mu in range(2):
                        nc.tensor.matmul(
                            out=psp,
                            lhsT=pw_sb[:, k, mu, lch],
                            rhs=_flat(h_full[:, mu]),
                            start=(mu == 0),
                            stop=(mu == 1),
                        )
                    nc.scalar.activation(
                        out=pkT[:, lch],
                        in_=psp,
                        func=AF.Identity,
                        bias=pb_sb[:, k, lch : lch + 1],
                        scale=1.0,
                    )
                # local-column P_k^T from this core's own h (bitwise-matching
                # values for the diagonal extraction)
                pkl = pkp.tile([128, 2, BL], F32, tag="pkl")
                if not SIMSUB & 1:
                    nc.vector.memset(pkl, 0.001)
                for lch in range(2 if SIMSUB & 1 else 0):
                    psp = ps_pk.tile([128, BL], F32, tag="ps_pk")
                    for mu in range(2):
                        nc.tensor.matmul(
                            out=psp,
                            lhsT=pw_sb[:, k, mu, lch],
                            rhs=h[:, mu],
                            start=(mu == 0),
                            stop=(mu == 1),
                        )
                    nc.scalar.activation(
                        out=pkl[:, lch],
                        in_=psp,
                        func=AF.Identity,
                        bias=pb_sb[:, k, lch : lch + 1],
                        scale=1.0,
                    )
                # diagonal logits: D[j, j'] = enc[s=T_IN+k, j] . pkl[:, j']
                psd = ps_dk.tile([BL, BL], F32, tag="ps_dk")
                for lch in range(2 if SIMSUB & 2 else 0):
                    nc.tensor.matmul(
                        out=psd,
                        lhsT=encT[:, lch, T_IN + k],
                        rhs=pkl[:, lch],
                        start=(lch == 0),
                        stop=(lch == 1),
                    )
                if SIMSUB & 2:
                    nc.vector.tensor_tensor(
                        out=dscr, in0=psd, in1=id_sb, op=ALU.mult
                    )
                    nc.vector.tensor_reduce(
                        out=diag_sb[:, k : k + 1],
                        in_=dscr,
                        axis=mybir.AxisListType.X,
                        op=ALU.add,
                    )

                # sim supers: rows in chunks of 3x512 (last 512,512,384)
                sizes = [(512, 512, 512), (512, 512, 512), (512, 512, 384)]
                for cch in range(2 if SIMSUB & 4 else 0):
                    for sup in range(3):
                        r0 = sup * 1536
                        sz = sizes[sup]
                        pss = ps_sim.tile([128, 3, 512], F32, tag="ps_sim")
                        for kch in range(2):
                            for j in range(3):
                                nc.tensor.matmul(
                                    out=pss[:, j, : sz[j]],
                                    lhsT=pkT[
                                        :, kch, cch * 128 : (cch + 1) * 128
                                    ],
                                    rhs=_flat(encT[:, kch])[
                                        :,
                                        r0
                                        + j * 512 : r0
                                        + j * 512
                                        + sz[j],
                                    ],
                                    start=(kch == 0),
                                    stop=(kch == 1),
                                )
                        for j in range(3):
                            idx = sup * 3 + j
                            nc.vector.tensor_reduce(
                                out=mpart[:, cch, k, idx : idx + 1],
                                in_=pss[:, j, : sz[j]],
                                axis=mybir.AxisListType.X,
                                op=ALU.max,
                            )
                            ed = dmp.tile([128, 512], BF16, tag="ed")
                            if SIMSUB & 8: nc.scalar.activation(
                                out=ed[:, : sz[j]],
                                in_=pss[:, j, : sz[j]],
                                func=AF.Exp,
                                bias=shift_sb[:, 0:1],
                                scale=1.0,
                                accum_out=spart[:, cch, k, idx : idx + 1],
                            )
                    nc.vector.tensor_reduce(
                        out=m_sb[:, cch, k : k + 1],
                        in_=mpart[:, cch, k],
                        axis=mybir.AxisListType.X,
                        op=ALU.max,
                    )
                    nc.vector.tensor_reduce(
                        out=s_sb[:, cch, k : k + 1],
                        in_=spart[:, cch, k],
                        axis=mybir.AxisListType.X,
                        op=ALU.add,
                    )

        nc.sync.dma_start(out=out_m.ap(), in_=m_sb)
        nc.sync.dma_start(out=out_s.ap(), in_=s_sb)
        nc.sync.dma_start(out=out_diag.ap(), in_=diag_sb)

    nc.compile()
    return nc


def _prep_inputs(X, conv_w, conv_b, W_ih, W_hh, b_ih, b_hh, pred_W, pred_b):
    X = np.ascontiguousarray(np.asarray(X, dtype=np.float32))
    conv_w = np.asarray(conv_w, dtype=np.float32)
    conv_b = np.asarray(conv_b, dtype=np.float32)
    W_ih = np.asarray(W_ih, dtype=np.float32)
    W_hh = np.asarray(W_hh, dtype=np.float32)
    b_ih = np.asarray(b_ih, dtype=np.float32)
    b_hh = np.asarray(b_hh, dtype=np.float32)
    pred_W = np.asarray(pred_W, dtype=np.float32)
    pred_b = np.asarray(pred_b, dtype=np.float32)

    wr = np.ascontiguousarray(conv_w.transpose(2, 1, 0).reshape(KW, H))
    cb = np.ascontiguousarray(conv_b.reshape(LCH, 128).T)
    wih = np.ascontiguousarray(
        W_ih.T.reshape(2, 128, GCH, 128).transpose(1, 0, 2, 3)
    )
    whh = np.ascontiguousarray(
        W_hh.T.reshape(2, 128, GCH, 128).transpose(1, 0, 2, 3)
    )
    gib_vec = b_ih.copy()
    gib_vec[: 2 * H] += b_hh[: 2 * H]
    gib = np.ascontiguousarray(gib_vec.reshape(GCH, 128).T)
    bhn = np.ascontiguousarray(b_hh[2 * H :].reshape(2, 128).T)
    pw = np.ascontiguousarray(
        pred_W.transpose(0, 2, 1)
        .reshape(T_OUT, 2, 128, 2, 128)
        .transpose(2, 0, 1, 3, 4)
    )
    pb = np.ascontiguousarray(pred_b.reshape(T_OUT, 2, 128).transpose(2, 0, 1))
    id32 = np.eye(BL, dtype=np.float32)

    shared = dict(
        wr=wr, cb=cb, wih=wih, whh=whh, gib=gib, bhn=bhn, pw=pw, pb=pb, id32=id32
    )
    in_maps = []
    for i in range(NCORES):
        xs = X[i * BL : (i + 1) * BL, : S * STRIDE, :]
        xT = np.ascontiguousarray(
            xs.reshape(BL, S, STRIDE, C).transpose(2, 3, 1, 0).reshape(KW, S, BL)
        )
        in_maps.append(dict(shared, xT=xT))
    return in_maps


def _combine(results):
    m_parts, s_parts, diags, hs = [], [], [], []
    for res in results:
        m_parts.append(
            np.asarray(res["out_m"]).transpose(2, 1, 0).reshape(T_OUT, B)
        )
        s_parts.append(
            np.asarray(res["out_s"]).transpose(2, 1, 0).reshape(T_OUT, B)
        )
        diags.append(np.asarray(res["out_diag"]).T)  # [T_OUT, BL]
        hs.append(
            np.asarray(res["out_h"]).transpose(2, 1, 0).reshape(BL, H)
        )
    m = np.max(np.stack(m_parts), axis=0)          # [T_OUT, B]
    s = np.sum(np.stack(s_parts, axis=0), axis=0, dtype=np.float32)
    diag = np.concatenate(diags, axis=1)           # [T_OUT, B]
    hidden = np.concatenate(hs, axis=0)[None]      # [1, B, H]

    lse = np.log(s, dtype=np.float32) - np.float32(EXP_SHIFT)
    loss = -np.sum(diag - lse, dtype=np.float32) / np.float32(T_OUT * B)
    correct = np.sum(diag == m)
    accuracy = np.float32(correct) / np.float32(T_OUT * B)
    return (
        np.asarray(accuracy, dtype=np.float32),
        np.asarray(loss, dtype=np.float32),
        hidden.astype(np.float32),
    )


def kernel(X, conv_w, conv_b, W_ih, W_hh, b_ih, b_hh, pred_W, pred_b, **kw):
    if "nc" not in _CACHE:
        _CACHE["nc"] = _build()
    nc = _CACHE["nc"]
    in_maps = _prep_inputs(
        X, conv_w, conv_b, W_ih, W_hh, b_ih, b_hh, pred_W, pred_b
    )
    res = run_bass_kernel_spmd(
        nc, in_maps, core_ids=list(range(NCORES)), **kw
    )
    out = _combine(res.results)
    _CACHE["last_results"] = res
    return out


# revision 16
# speedup vs baseline: 1.0368x; 1.0368x over previous
"""CPC loss kernel for Trainium2, 8 NeuronCores, batch-sharded SPMD.

Pipeline per core (batch shard of 32):
  conv1d (as matmul over im2col'd input) -> enc_T [256, 140, 32] in SBUF
  GRU 128 steps (gate-on-partition layout; input projection gi precomputed
  in 32-step chunks that fill PE gaps during the recurrence)
  AllGather h_last across the 8 cores
  per prediction head k: pred matmul, then sim matmuls (rows = local
  (s, b) pairs, all 256 contrastive columns) fused with softmax stats:
  DVE max-reduce + ACT exp(accum_out) straight from PSUM.
The conv/GRU path runs in fp32 (hidden state is graded tightly); the
contrastive phase runs the matmuls in float32r (4x PE throughput,
~1.6e-4 relative error, far inside the loss tolerance).
Outputs per core: max/sumexp partials over local rows, diagonal logits,
hidden-state shard. Final (accuracy, loss, hidden) assembled on host.
"""

import os
import sys

sys.path.insert(0, "/opt/trn_rl_repo")

import numpy as np

import concourse.bass as bass
import concourse.tile as tile
from concourse import bacc, mybir
from concourse.bass_utils import run_bass_kernel_spmd

F32 = mybir.dt.float32
F32R = mybir.dt.float32r
BF16 = mybir.dt.bfloat16
AF = mybir.ActivationFunctionType
ALU = mybir.AluOpType

NCORES = 8
B, L, C = 256, 2048, 12
T_IN, T_OUT, H, STRIDE = 128, 12, 256, 4
S = T_IN + T_OUT              # 140 steps used
BL = B // NCORES              # 32 batch per core
ROWS = S * BL                 # 4480 local sim rows
KW = C * STRIDE               # 48 im2col contraction
G = 3 * H                     # 768 gate width
GCH = G // 128                # 6 gate chunks
LCH = H // 128                # 2 latent chunks
EXP_SHIFT = -20.0             # exp(x + EXP_SHIFT); added back on host

PHASES = int(os.environ.get("KERNEL_PHASES", "4"))  # debug bisect knob
NO_CC = int(os.environ.get("KERNEL_NO_CC", "0"))    # skip collective (TimelineSim)

_CACHE = {}


def _flat(ap):
    return ap.rearrange("p a b -> p (a b)")


def _build(zero_bias=True):
    nc = bacc.Bacc(
        "TRN2",
        target_bir_lowering=False,
        debug=False,
        enable_asserts=False,
        num_devices=NCORES,
    )

    # ---- per-core inputs (host pre-laid-out, see kernel()) ----
    xT = nc.dram_tensor("xT", [KW, S, BL], F32, kind="ExternalInput")
    wr = nc.dram_tensor("wr", [KW, H], F32, kind="ExternalInput")
    cb = nc.dram_tensor("cb", [128, LCH], F32, kind="ExternalInput")
    wih = nc.dram_tensor("wih", [128, 2, GCH, 128], F32, kind="ExternalInput")
    whh = nc.dram_tensor("whh", [128, 2, GCH, 128], F32, kind="ExternalInput")
    gib = nc.dram_tensor("gib", [128, GCH], F32, kind="ExternalInput")
    bhn = nc.dram_tensor("bhn", [128, 2], F32, kind="ExternalInput")
    pw = nc.dram_tensor("pw", [128, T_OUT, 2, 2, 128], F32R, kind="ExternalInput")
    pb = nc.dram_tensor("pb", [128, T_OUT, 2], F32, kind="ExternalInput")
    id32 = nc.dram_tensor("id32", [BL, BL], F32, kind="ExternalInput")

    # ---- per-core outputs ----
    out_m = nc.dram_tensor("out_m", [128, 2, T_OUT], F32, kind="ExternalOutput")
    out_s = nc.dram_tensor("out_s", [128, 2, T_OUT], F32, kind="ExternalOutput")
    out_diag = nc.dram_tensor("out_diag", [BL, T_OUT], F32, kind="ExternalOutput")
    out_h = nc.dram_tensor("out_h", [128, LCH, BL], F32, kind="ExternalOutput")

    h_gath = nc.dram_tensor(
        "h_gath", [NCORES, 128, LCH, BL], F32R, addr_space="Shared"
    )

    from contextlib import ExitStack

    with tile.TileContext(nc) as tc, ExitStack() as ctx:
        consts = ctx.enter_context(tc.tile_pool(name="consts", bufs=1))
        bigs = ctx.enter_context(tc.tile_pool(name="bigs", bufs=1))
        gip = ctx.enter_context(tc.tile_pool(name="gip", bufs=2))
        hp = ctx.enter_context(tc.tile_pool(name="hp", bufs=3))
        gp = ctx.enter_context(tc.tile_pool(name="gates", bufs=3))
        stp = ctx.enter_context(tc.tile_pool(name="stats", bufs=1))
        pkp = ctx.enter_context(tc.tile_pool(name="pkp", bufs=2))
        dmp = ctx.enter_context(tc.tile_pool(name="dump", bufs=2))
        drp = ctx.enter_context(tc.tile_pool(name="dram", bufs=1, space="DRAM"))

        # ---- load constants ----
        wr_sb = consts.tile([KW, H], F32, tag="wr")
        nc.sync.dma_start(out=wr_sb, in_=wr.ap())
        cb_sb = consts.tile([128, LCH], F32, tag="cb")
        nc.sync.dma_start(out=cb_sb, in_=cb.ap())
        wih_sb = consts.tile([128, 2, GCH, 128], F32, tag="wih")
        nc.sync.dma_start(out=wih_sb, in_=wih.ap())
        whh_sb = consts.tile([128, 2, GCH, 128], F32, tag="whh")
        nc.sync.dma_start(out=whh_sb, in_=whh.ap())
        gib_sb = consts.tile([128, GCH], F32, tag="gib")
        nc.sync.dma_start(out=gib_sb, in_=gib.ap())
        bhn_sb = consts.tile([128, 2], F32, tag="bhn")
        nc.sync.dma_start(out=bhn_sb, in_=bhn.ap())
        pw_sb = consts.tile([128, T_OUT, 2, 2, 128], F32R, tag="pw")
        nc.sync.dma_start(out=pw_sb, in_=pw.ap())
        pb_sb = consts.tile([128, T_OUT, 2], F32, tag="pb")
        nc.sync.dma_start(out=pb_sb, in_=pb.ap())
        id_sb = consts.tile([BL, BL], F32, tag="id32")
        nc.sync.dma_start(out=id_sb, in_=id32.ap())
        shift_sb = consts.tile([128, 1], F32, tag="shift")
        nc.vector.memset(shift_sb, EXP_SHIFT)

        encT = bigs.tile([128, LCH, S, BL], F32, tag="encT")
        enc_r = bigs.tile([128, LCH, S, BL], F32R, tag="enc_r")

        with tc.tile_pool(name="ps_a", bufs=2, space="PSUM") as ps_a, tc.tile_pool(
            name="ps_g", bufs=2, space="PSUM"
        ) as ps_g:
            # ---- conv: enc_T[lat, (s, b)] = wr.T @ x ----
            with tc.tile_pool(name="xp", bufs=1) as xp:
                x_sb = xp.tile([KW, S * BL], F32, tag="x")
                nc.sync.dma_start(
                    out=x_sb, in_=xT.ap().rearrange("p s b -> p (s b)")
                )
                for mch in range(LCH):
                    for j0 in range(0, S * BL, 512):
                        n = min(512, S * BL - j0)
                        ps = ps_a.tile([128, 512], F32, tag="ps_a")
                        nc.tensor.matmul(
                            out=ps[:, :n],
                            lhsT=wr_sb[:, mch * 128 : (mch + 1) * 128],
                            rhs=x_sb[:, j0 : j0 + n],
                            start=True,
                            stop=True,
                        )
                        nc.scalar.activation(
                            out=_flat(encT[:, mch])[:, j0 : j0 + n],
                            in_=ps[:, :n],
                            func=AF.Identity,
                            bias=cb_sb[:, mch : mch + 1],
                            scale=1.0,
                        )

            # ---- gi chunks: gi[g, ts, b] = W_ih.T-tiles @ enc chunk ----
            NGI = T_IN // 32  # 4 chunks of 32 steps
            gi_tiles = [None] * NGI

            def gi_chunk_gen(tch):
                t0 = tch * 32
                gt = gip.tile([128, GCH, 32, BL], F32, tag="gi")
                gi_tiles[tch] = gt
                for gch in range(GCH):
                    ps0 = ps_a.tile([128, 512], F32, tag="ps_a")
                    ps1 = ps_a.tile([128, 512], F32, tag="ps_a")
                    for kch in range(2):
                        nc.tensor.matmul(
                            out=ps0,
                            lhsT=wih_sb[:, kch, gch],
                            rhs=_flat(encT[:, kch, t0 : t0 + 16]),
                            start=(kch == 0),
                            stop=(kch == 1),
                        )
                        yield
                        nc.tensor.matmul(
                            out=ps1,
                            lhsT=wih_sb[:, kch, gch],
                            rhs=_flat(encT[:, kch, t0 + 16 : t0 + 32]),
                            start=(kch == 0),
                            stop=(kch == 1),
                        )
                        yield
                    bias = 0.0 if zero_bias else gib_sb[:, gch : gch + 1]
                    nc.scalar.activation(
                        out=_flat(gt[:, gch, 0:16]),
                        in_=ps0,
                        func=AF.Identity,
                        bias=bias,
                        scale=1.0,
                    )
                    yield
                    nc.scalar.activation(
                        out=_flat(gt[:, gch, 16:32]),
                        in_=ps1,
                        func=AF.Identity,
                        bias=bias,
                        scale=1.0,
                    )
                    yield

            # chunk 0 up front; chunk c+1 drip-fed during chunk c's steps
            for _ in gi_chunk_gen(0):
                pass

            # enc_r (float32r copy for the contrastive phase), drip-fed
            # into GRU idle time on the DVE
            def enc_r_gen():
                for lch in range(LCH):
                    flat_src = _flat(encT[:, lch])
                    flat_dst = _flat(enc_r[:, lch])
                    for j0 in range(0, S * BL, 1120):
                        n = min(1120, S * BL - j0)
                        nc.vector.tensor_copy(
                            out=flat_dst[:, j0 : j0 + n],
                            in_=flat_src[:, j0 : j0 + n],
                        )
                        yield

            # ---- GRU recurrence ----
            h = hp.tile([128, LCH, BL], F32, tag="h")
            nc.vector.memset(h, 0.0)
            feeders = [enc_r_gen()]
            if PHASES >= 2:
                feeders.insert(0, gi_chunk_gen(1))
            for t in range(T_IN if PHASES >= 2 else 0):
                tch, ts = t // 32, t % 32
                if ts == 0 and 2 <= tch + 1 < NGI:
                    feeders.insert(0, gi_chunk_gen(tch + 1))
                gt = gi_tiles[tch]

                psg = ps_g.tile([128, GCH, BL], F32, tag="ps_g")
                for gch in (0, 1, 2, 3, 4, 5):
                    for kch in range(2):
                        nc.tensor.matmul(
                            out=psg[:, gch],
                            lhsT=whh_sb[:, kch, gch],
                            rhs=h[:, kch],
                            start=(kch == 0),
                            stop=(kch == 1),
                        )
                # r|z = sigmoid(gi_rz + gh_rz)  (one wide op pair)
                a_rz = gp.tile([128, 4, BL], F32, tag="a_rz")
                nc.vector.tensor_tensor(
                    out=a_rz, in0=psg[:, 0:4], in1=gt[:, 0:4, ts], op=ALU.add
                )
                rz = gp.tile([128, 4, BL], F32, tag="rz")
                nc.scalar.activation(out=rz, in_=a_rz, func=AF.Sigmoid)
                # n = tanh(gi_n + r * (gh_n + b_hh_n))
                rhn = gp.tile([128, 2, BL], F32, tag="rhn")
                if zero_bias:
                    nc.vector.tensor_tensor(
                        out=rhn, in0=psg[:, 4:6], in1=rz[:, 0:2], op=ALU.mult
                    )
                else:
                    for ch in range(2):
                        nc.vector.scalar_tensor_tensor(
                            out=rhn[:, ch],
                            in0=psg[:, 4 + ch],
                            scalar=bhn_sb[:, ch : ch + 1],
                            in1=rz[:, ch],
                            op0=ALU.add,
                            op1=ALU.mult,
                        )
                gn = gp.tile([128, 2, BL], F32, tag="gn")
                nc.vector.tensor_tensor(
                    out=gn, in0=rhn, in1=gt[:, 4:6, ts], op=ALU.add
                )
                n = gp.tile([128, 2, BL], F32, tag="n")
                nc.scalar.activation(out=n, in_=gn, func=AF.Tanh)
                # h' = n + z * (h - n)
                d = gp.tile([128, 2, BL], F32, tag="d")
                nc.vector.tensor_tensor(out=d, in0=h, in1=n, op=ALU.subtract)
                e = gp.tile([128, 2, BL], F32, tag="e")
                nc.vector.tensor_tensor(out=e, in0=rz[:, 2:4], in1=d, op=ALU.mult)
                h_new = hp.tile([128, LCH, BL], F32, tag="h")
                nc.vector.tensor_tensor(out=h_new, in0=e, in1=n, op=ALU.add)
                h = h_new

                # drip-feed deferred work into the stream
                budget = 2
                while budget > 0 and feeders:
                    if next(feeders[0], "done") == "done":
                        feeders.pop(0)
                    else:
                        budget -= 1
            # drain leftover feeder work (enc_r tail etc.)
            for f in feeders:
                for _ in f:
                    pass

        # ---- gather h_last across cores ----
        nc.sync.dma_start(out=out_h.ap(), in_=h)
        h_r = stp.tile([128, LCH, BL], F32R, tag="h_r")
        nc.vector.tensor_copy(out=h_r, in_=h)
        h_full = bigs.tile([128, LCH, NCORES, BL], F32R, tag="h_full")
        if PHASES >= 3 and not NO_CC:
            h_loc = drp.tile([128, LCH, BL], F32R, tag="h_loc")
            nc.gpsimd.dma_start(out=h_loc, in_=h_r)
            nc.gpsimd.collective_compute(
                "AllGather",
                ALU.bypass,
                replica_groups=[list(range(NCORES))],
                ins=[h_loc.opt()],
                outs=[h_gath.ap()],
            )
            nc.sync.dma_start(
                out=h_full, in_=h_gath.ap().rearrange("c p l b -> p l c b")
            )
        else:
            nc.vector.memset(h_full, 0.0)

        # ---- per-head pred + sim + softmax stats ----
        NSUP = 3  # row supers of 3x512 (last 512,512,384)
        NPART = 4  # 2 full supers + split tail (2x512 | 384)
        mpart = stp.tile([128, 2, T_OUT, NPART], F32, tag="mpart")
        spart = stp.tile([128, 2, T_OUT, NPART], F32, tag="spart")
        m_sb = stp.tile([128, 2, T_OUT], F32, tag="m_sb")
        s_sb = stp.tile([128, 2, T_OUT], F32, tag="s_sb")
        diag_sb = stp.tile([BL, T_OUT], F32, tag="diag")
        dscr = stp.tile([BL, BL], F32, tag="dscr")

        nc.vector.memset(m_sb, 0.0)
        nc.vector.memset(s_sb, 1.0)
        nc.vector.memset(diag_sb, 0.0)

        with tc.tile_pool(name="ps_sim", bufs=2, space="PSUM") as ps_sim, tc.tile_pool(
            name="ps_pk", bufs=1, space="PSUM"
        ) as ps_pk, tc.tile_pool(name="ps_dk", bufs=1, space="PSUM") as ps_dk:
            for k in range(T_OUT if PHASES >= 4 else 0):
                # P_k^T [lam, c] for all 256 c
                pkT = pkp.tile([128, 2, B], F32R, tag="pkT")
                for lch in range(2):
                    psp = ps_pk.tile([128, B], F32, tag="ps_pk")
                    for mu in range(2):
                        nc.tensor.matmul(
                            out=psp,
                            lhsT=pw_sb[:, k, mu, lch],
                            rhs=_flat(h_full[:, mu]),
                            start=(mu == 0),
                            stop=(mu == 1),
                        )
                    nc.scalar.activation(
                        out=pkT[:, lch],
                        in_=psp,
                        func=AF.Identity,
                        bias=pb_sb[:, k, lch : lch + 1],
                        scale=1.0,
                    )
                # local-column P_k^T from this core's own h (values identical
                # to the corresponding pkT columns)
                pkl = pkp.tile([128, 2, BL], F32R, tag="pkl")
                for lch in range(2):
                    psp = ps_pk.tile([128, BL], F32, tag="ps_pk")
                    for mu in range(2):
                        nc.tensor.matmul(
                            out=psp,
                            lhsT=pw_sb[:, k, mu, lch],
                            rhs=h_r[:, mu],
                            start=(mu == 0),
                            stop=(mu == 1),
                        )
                    nc.scalar.activation(
                        out=pkl[:, lch],
                        in_=psp,
                        func=AF.Identity,
                        bias=pb_sb[:, k, lch : lch + 1],
                        scale=1.0,
                    )
                # diagonal logits: D[j, j'] = enc[s=T_IN+k, j] . pkl[:, j']
                psd = ps_dk.tile([BL, BL], F32, tag="ps_dk")
                for lch in range(2):
                    nc.tensor.matmul(
                        out=psd,
                        lhsT=enc_r[:, lch, T_IN + k],
                        rhs=pkl[:, lch],
                        start=(lch == 0),
                        stop=(lch == 1),
                    )
                nc.vector.tensor_tensor(
                    out=dscr, in0=psd, in1=id_sb, op=ALU.mult
                )
                nc.vector.tensor_reduce(
                    out=diag_sb[:, k : k + 1],
                    in_=dscr,
                    axis=mybir.AxisListType.X,
                    op=ALU.add,
                )

                # sim supers: rows in chunks of 3x512 (last 512,512,384)
                for cch in range(2):
                    for sup in range(NSUP):
                        r0 = sup * 1536
                        sz = (512, 512, 512) if sup < 2 else (512, 512, 384)
                        pss = ps_sim.tile([128, 3, 512], F32, tag="ps_sim")
                        for kch in range(2):
                            for j in range(3):
                                nc.tensor.matmul(
                                    out=pss[:, j, : sz[j]],
                                    lhsT=pkT[:, kch, cch * 128 : (cch + 1) * 128],
                                    rhs=_flat(enc_r[:, kch])[
                                        :, r0 + j * 512 : r0 + j * 512 + sz[j]
                                    ],
                                    start=(kch == 0),
                                    stop=(kch == 1),
                                )
                        ed = dmp.tile([128, 3, 512], BF16, tag="ed")
                        if sup < 2:
                            windows = [(pss, ed, sup)]
                        else:
                            windows = [
                                (pss[:, 0:2], ed[:, 0:2], 2),
                                (pss[:, 2, :384], ed[:, 2, :384], 3),
                            ]
                        for win_in, win_out, slot in windows:
                            nc.vector.tensor_reduce(
                                out=mpart[:, cch, k, slot : slot + 1],
                                in_=win_in,
                                axis=(
                                    mybir.AxisListType.XY
                                    if len(win_in.shape) > 2
                                    else mybir.AxisListType.X
                                ),
                                op=ALU.max,
                            )
                            nc.scalar.activation(
                                out=win_out,
                                in_=win_in,
                                func=AF.Exp,
                                bias=shift_sb[:, 0:1],
                                scale=1.0,
                                accum_out=spart[:, cch, k, slot : slot + 1],
                            )
                    nc.vector.tensor_reduce(
                        out=m_sb[:, cch, k : k + 1],
                        in_=mpart[:, cch, k],
                        axis=mybir.AxisListType.X,
                        op=ALU.max,
                    )
                    nc.vector.tensor_reduce(
                        out=s_sb[:, cch, k : k + 1],
                        in_=spart[:, cch, k],
                        axis=mybir.AxisListType.X,
                        op=ALU.add,
                    )

        nc.sync.dma_start(out=out_m.ap(), in_=m_sb)
        nc.sync.dma_start(out=out_s.ap(), in_=s_sb)
        nc.sync.dma_start(out=out_diag.ap(), in_=diag_sb)

    nc.compile()
    return nc


def _prep_inputs(X, conv_w, conv_b, W_ih, W_hh, b_ih, b_hh, pred_W, pred_b):
    X = np.ascontiguousarray(np.asarray(X, dtype=np.float32))
    conv_w = np.asarray(conv_w, dtype=np.float32)
    conv_b = np.asarray(conv_b, dtype=np.float32)
    W_ih = np.asarray(W_ih, dtype=np.float32)
    W_hh = np.asarray(W_hh, dtype=np.float32)
    b_ih = np.asarray(b_ih, dtype=np.float32)
    b_hh = np.asarray(b_hh, dtype=np.float32)
    pred_W = np.asarray(pred_W, dtype=np.float32)
    pred_b = np.asarray(pred_b, dtype=np.float32)

    wr = np.ascontiguousarray(conv_w.transpose(2, 1, 0).reshape(KW, H))
    cb = np.ascontiguousarray(conv_b.reshape(LCH, 128).T)
    wih = np.ascontiguousarray(
        W_ih.T.reshape(2, 128, GCH, 128).transpose(1, 0, 2, 3)
    )
    whh = np.ascontiguousarray(
        W_hh.T.reshape(2, 128, GCH, 128).transpose(1, 0, 2, 3)
    )
    gib_vec = b_ih.copy()
    gib_vec[: 2 * H] += b_hh[: 2 * H]
    gib = np.ascontiguousarray(gib_vec.reshape(GCH, 128).T)
    bhn = np.ascontiguousarray(b_hh[2 * H :].reshape(2, 128).T)
    pw = np.ascontiguousarray(
        pred_W.transpose(0, 2, 1)
        .reshape(T_OUT, 2, 128, 2, 128)
        .transpose(2, 0, 1, 3, 4)
    )
    pb = np.ascontiguousarray(pred_b.reshape(T_OUT, 2, 128).transpose(2, 0, 1))
    id32 = np.eye(BL, dtype=np.float32)

    zero_bias = not (np.any(b_ih) or np.any(b_hh))

    shared = dict(
        wr=wr, cb=cb, wih=wih, whh=whh, gib=gib, bhn=bhn, pw=pw, pb=pb, id32=id32
    )
    in_maps = []
    for i in range(NCORES):
        xs = X[i * BL : (i + 1) * BL, : S * STRIDE, :]
        xTl = np.ascontiguousarray(
            xs.reshape(BL, S, STRIDE, C).transpose(2, 3, 1, 0).reshape(KW, S, BL)
        )
        in_maps.append(dict(shared, xT=xTl))
    return in_maps, zero_bias


def _combine(results):
    m_parts, s_parts, diags, hs = [], [], [], []
    for res in results:
        m_parts.append(
            np.asarray(res["out_m"]).transpose(2, 1, 0).reshape(T_OUT, B)
        )
        s_parts.append(
            np.asarray(res["out_s"]).transpose(2, 1, 0).reshape(T_OUT, B)
        )
        diags.append(np.asarray(res["out_diag"]).T)  # [T_OUT, BL]
        hs.append(np.asarray(res["out_h"]).transpose(2, 1, 0).reshape(BL, H))
    m = np.max(np.stack(m_parts), axis=0)          # [T_OUT, B]
    s = np.sum(np.stack(s_parts, axis=0), axis=0, dtype=np.float32)
    diag = np.concatenate(diags, axis=1)           # [T_OUT, B]
    hidden = np.concatenate(hs, axis=0)[None]      # [1, B, H]

    lse = np.log(s, dtype=np.float32) - np.float32(EXP_SHIFT)
    loss = -np.sum(diag - lse, dtype=np.float32) / np.float32(T_OUT * B)
    correct = np.sum(diag == m)
    accuracy = np.float32(correct) / np.float32(T_OUT * B)
    return (
        np.asarray(accuracy, dtype=np.float32),
        np.asarray(loss, dtype=np.float32),
        hidden.astype(np.float32),
    )


def kernel(X, conv_w, conv_b, W_ih, W_hh, b_ih, b_hh, pred_W, pred_b, **kw):
    in_maps, zero_bias = _prep_inputs(
        X, conv_w, conv_b, W_ih, W_hh, b_ih, b_hh, pred_W, pred_b
    )
    key = ("nc", zero_bias)
    if key not in _CACHE:
        _CACHE[key] = _build(zero_bias)
    nc = _CACHE[key]
    res = run_bass_kernel_spmd(nc, in_maps, core_ids=list(range(NCORES)), **kw)
    out = _combine(res.results)
    _CACHE["last_results"] = res
    return out
